# revision 1
# baseline (speedup 1.0000x reference)
"""Distributed causal attention head for Trainium2 (8 NeuronCores).

Reference computation (single head):
  q = x @ Wq.T + bq ; k = x @ Wk.T + bk ; v = x @ Wv.T + bv
  scores = (q @ k.T) / sqrt(D)  with causal mask, softmax, out = attn @ v

Sharding: 16 query blocks of S/16 rows; core c owns blocks (c, 15-c) so causal
work is balanced. Each core projects q/k/v for its own rows (bf16 matmuls,
fp32 PSUM), all-gathers k^T/v across the 8 cores, then runs its causal
attention schedule (statically specialized per core inside tc.If branches on
the partition id). Scores are computed transposed ([key, query] layout) so
softmax normalization comes from an extra all-ones column appended to V and
no on-chip transposes are needed.
"""
import sys

sys.path.insert(0, "/opt/trn_rl_repo")

import numpy as np
import concourse.bass as bass
import concourse.mybir as mybir
from concourse import bacc, tile
from concourse.bass_utils import run_bass_kernel_spmd

F32 = mybir.dt.float32
BF16 = mybir.dt.bfloat16
N_CORES = 8
N_BLOCKS = 16
H = 256
HC = 2  # head-dim chunks of 128


def _owner(g):
    """Global block g -> (owner core, position within owner's pair)."""
    return (g, 0) if g < N_CORES else (N_BLOCKS - 1 - g, 1)


def build_nc(S=4096, D=2048):
    """Build the SPMD graph (same on all 8 cores)."""
    BLK = S // N_BLOCKS          # query block rows (256 full-size)
    CH = BLK // 2                # key chunk rows (128 full-size)
    SLOC = 2 * BLK               # rows per core (512 full-size)
    KC = D // 128                # contraction chunks
    KT_ELEMS = HC * 128 * SLOC   # per-core kT contribution elems
    V_ELEMS = 4 * CH * (H + 1)   # per-core v contribution elems
    CONTRIB = KT_ELEMS + V_ELEMS
    SCALE = 1.0 / float(np.sqrt(D))
    assert CH <= 128

    nc = bacc.Bacc("TRN2", target_bir_lowering=False, debug=False,
                   enable_asserts=True, num_devices=N_CORES)
    # x and the weight matrices are pre-cast to bf16 on the host (identical
    # numerics to an on-chip cast, half the HBM traffic); biases stay f32.
    xT = nc.dram_tensor("xT", [D, SLOC], BF16, kind="ExternalInput")
    wqT = nc.dram_tensor("wqT", [D, H], BF16, kind="ExternalInput")
    wkT = nc.dram_tensor("wkT", [D, H], BF16, kind="ExternalInput")
    wvT = nc.dram_tensor("wvT", [D, H], BF16, kind="ExternalInput")
    bq = nc.dram_tensor("bq", [1, H], F32, kind="ExternalInput")
    bk = nc.dram_tensor("bk", [1, H], F32, kind="ExternalInput")
    bv = nc.dram_tensor("bv", [1, H], BF16, kind="ExternalInput")
    out = nc.dram_tensor("out", [SLOC, H], F32, kind="ExternalOutput")

    with tile.TileContext(nc) as tc:
        with (
            tc.tile_pool(name="big", bufs=1) as big,
            tc.tile_pool(name="small", bufs=1) as small,
            tc.tile_pool(name="pt", bufs=10) as ptpool,
            tc.tile_pool(name="osb", bufs=4) as osbpool,
            tc.tile_pool(name="psum", bufs=3, space="PSUM") as psum,
            tc.tile_pool(name="psum_o", bufs=1, space="PSUM") as psum_o_pool,
            tc.tile_pool(name="dram", bufs=1, space="DRAM") as dram,
        ):
            # ---- inputs -> SBUF (already bf16 from the host) ----
            # Split per 4 contraction chunks so the first projection matmuls
            # start as soon as the leading slices land (-12us on the cost
            # model vs one monolithic DMA per tensor).
            xT_sb = big.tile([128, KC, SLOC], BF16, tag="xT")
            wk_sb = big.tile([128, KC, H], BF16, tag="wk")
            wv_sb = big.tile([128, KC, H], BF16, tag="wv")
            wq_sb = big.tile([128, KC, H], BF16, tag="wq")
            xv = xT[:].rearrange("(kc p) s -> p kc s", p=128)
            wkv = wkT[:].rearrange("(kc p) h -> p kc h", p=128)
            wvv = wvT[:].rearrange("(kc p) h -> p kc h", p=128)
            wqv = wqT[:].rearrange("(kc p) h -> p kc h", p=128)
            for k0 in range(0, KC, 4):
                k1 = min(k0 + 4, KC)
                nc.sync.dma_start(wk_sb[:, k0:k1, :], wkv[:, k0:k1, :])
                nc.sync.dma_start(xT_sb[:, k0:k1, :], xv[:, k0:k1, :])
                nc.sync.dma_start(wv_sb[:, k0:k1, :], wvv[:, k0:k1, :])
                nc.sync.dma_start(wq_sb[:, k0:k1, :], wqv[:, k0:k1, :])
            bq_sb = small.tile([128, HC], F32, tag="bq")
            bk_sb = small.tile([128, HC], F32, tag="bk")
            nc.sync.dma_start(bq_sb[:], bq[0, :].rearrange("(hc p) -> p hc", p=128))
            nc.sync.dma_start(bk_sb[:], bk[0, :].rearrange("(hc p) -> p hc", p=128))
            bv1_sb = small.tile([1, H + 1], BF16, tag="bv")
            nc.sync.dma_start(bv1_sb[:, 0:H], bv[:])
            nc.vector.memset(bv1_sb[:, H:H + 1], 1.0)
            ones_row = small.tile([1, SLOC], BF16, tag="ones")
            nc.vector.memset(ones_row[:], 1.0)

            # ---- k/v projections first (feed the collective asap) ----
            kT_sb = big.tile([128, HC, SLOC], BF16, tag="kT")
            for hc in range(HC):
                ps = psum.tile([128, SLOC], F32, tag="ps")
                for kc in range(KC):
                    nc.tensor.matmul(ps[:], wk_sb[:, kc, hc * 128:(hc + 1) * 128],
                                     xT_sb[:, kc, :], start=(kc == 0), stop=(kc == KC - 1))
                nc.vector.tensor_scalar_add(kT_sb[:, hc, :], ps[:], bk_sb[:, hc:hc + 1])

            v_sb = big.tile([CH, 4, H + 1], BF16, tag="v")
            for sc in range(4):
                ps = psum.tile([CH, H + 1], F32, tag="ps")
                nc.tensor.matmul(ps[:], ones_row[0:1, 0:CH], bv1_sb[:],
                                 start=True, stop=False)
                for kc in range(KC):
                    nc.tensor.matmul(ps[:, 0:H], xT_sb[:, kc, sc * CH:(sc + 1) * CH],
                                     wv_sb[:, kc, :], start=False, stop=(kc == KC - 1))
                nc.vector.tensor_copy(v_sb[:, sc, :], ps[:])

            # ---- bounce k/v to DRAM and all-gather ----
            cc_in = dram.tile([CONTRIB], BF16, tag="cc_in")
            cc_out = dram.tile([N_CORES * CONTRIB], BF16, tag="cc_out",
                               addr_space="Shared")
            nc.sync.dma_start(
                cc_in[0:KT_ELEMS].rearrange("(hc p s) -> p hc s", hc=HC, p=128, s=SLOC),
                kT_sb[:])
            nc.sync.dma_start(
                cc_in[KT_ELEMS:CONTRIB].rearrange("(sc p h) -> p sc h", sc=4, p=CH, h=H + 1),
                v_sb[:])
            nc.gpsimd.collective_compute(
                "AllGather", mybir.AluOpType.bypass,
                replica_groups=[list(range(N_CORES))],
                ins=[cc_in.opt()], outs=[cc_out.opt()],
            )

            # ---- q projection + diagonal-block scores overlap the collective ----
            qT_sb = big.tile([128, HC, SLOC], BF16, tag="qT")
            for hc in range(HC):
                ps = psum.tile([128, SLOC], F32, tag="ps")
                for kc in range(KC):
                    nc.tensor.matmul(ps[:], wq_sb[:, kc, hc * 128:(hc + 1) * 128],
                                     xT_sb[:, kc, :], start=(kc == 0), stop=(kc == KC - 1))
                nc.vector.tensor_scalar_add(qT_sb[:, hc, :], ps[:], bq_sb[:, hc:hc + 1])

            # Diagonal (own-block) scores: chunk m of own kT vs own q half.
            pt_diag = []
            for m in range(4):
                half = 0 if m < 2 else 1
                ps = psum.tile([CH, BLK], F32, tag="ps")
                for hc in range(HC):
                    nc.tensor.matmul(ps[:], kT_sb[:, hc, m * CH:(m + 1) * CH],
                                     qT_sb[:, hc, half * BLK:(half + 1) * BLK],
                                     start=(hc == 0), stop=(hc == HC - 1))
                pt = big.tile([CH, BLK], BF16, tag=f"ptd{m}")
                nc.scalar.activation(pt[:], ps[:], mybir.ActivationFunctionType.Exp,
                                     scale=SCALE)
                # keep iff si_local - sj_local >= 0 (si = y, sj = (m%2)*CH + x)
                nc.gpsimd.affine_select(
                    out=pt[:], in_=pt[:], compare_op=mybir.AluOpType.is_ge,
                    fill=0.0, base=-(m % 2) * CH,
                    pattern=[[1, BLK]], channel_multiplier=-1)
                pt_diag.append(pt)

            # Own left block vs right-half queries (always fully kept: block
            # c < block 15-c). Uniform across cores, overlaps the collective.
            pt_locr = []
            for u in range(2):
                ps = psum.tile([CH, BLK], F32, tag="ps")
                for hc in range(HC):
                    nc.tensor.matmul(ps[:], kT_sb[:, hc, u * CH:(u + 1) * CH],
                                     qT_sb[:, hc, BLK:2 * BLK],
                                     start=(hc == 0), stop=(hc == HC - 1))
                pt = big.tile([CH, BLK], BF16, tag=f"ptl{u}")
                nc.scalar.activation(pt[:], ps[:], mybir.ActivationFunctionType.Exp,
                                     scale=SCALE)
                pt_locr.append(pt)

            # output accumulators (per si chunk of CH rows)
            psum_o = [psum_o_pool.tile([CH, H + 1], F32, tag=f"out{sc}",
                                       name=f"psum_o{sc}")
                      for sc in range(4)]

            # ---- per-core causal schedule ----
            nc.cache_partition_id()
            pid = nc.partition_id()

            for c in range(N_CORES):
                with tc.If(pid == c):
                    lb, rb = c, N_BLOCKS - 1 - c
                    # gathered blocks needed beyond the diagonal ones
                    needed = [g for g in range(N_BLOCKS)
                              if (g < lb or g < rb) and g not in (lb, rb)]
                    nblk = len(needed)
                    kT_all = big.tile([128, HC, max(nblk, 1) * BLK], BF16, tag="kT_all")
                    v_all = big.tile([CH, max(2 * nblk, 1), H + 1], BF16, tag="v_all")
                    for i, g in enumerate(needed):
                        o, pos = _owner(g)
                        kt_view = cc_out[o * CONTRIB:o * CONTRIB + KT_ELEMS].rearrange(
                            "(hc p s) -> hc p s", hc=HC, p=128, s=SLOC)
                        nc.sync.dma_start(
                            kT_all[:, :, i * BLK:(i + 1) * BLK],
                            kt_view[:, :, pos * BLK:(pos + 1) * BLK].rearrange(
                                "hc p s -> p hc s"))
                        v_view = cc_out[o * CONTRIB + KT_ELEMS:(o + 1) * CONTRIB].rearrange(
                            "(sc p h) -> sc p h", sc=4, p=CH, h=H + 1)
                        nc.sync.dma_start(
                            v_all[:, 2 * i:2 * i + 2, :],
                            v_view[2 * pos:2 * pos + 2, :, :].rearrange("sc p h -> p sc h"))

                    # per-si-chunk contribution lists: (pT tile, col offset, v tile idx)
                    contribs = [[] for _ in range(4)]
                    # diagonal contributions (own blocks, pt_diag / local v_sb)
                    contribs[0].append((pt_diag[0], 0, ("loc", 0)))
                    contribs[1].append((pt_diag[0], CH, ("loc", 0)))
                    contribs[1].append((pt_diag[1], CH, ("loc", 1)))
                    contribs[2].append((pt_diag[2], 0, ("loc", 2)))
                    contribs[3].append((pt_diag[2], CH, ("loc", 2)))
                    contribs[3].append((pt_diag[3], CH, ("loc", 3)))
                    # own left block fully keyed by right-half queries
                    contribs[2].append((pt_locr[0], 0, ("loc", 0)))
                    contribs[2].append((pt_locr[1], 0, ("loc", 1)))
                    contribs[3].append((pt_locr[0], CH, ("loc", 0)))
                    contribs[3].append((pt_locr[1], CH, ("loc", 1)))

                    # gathered blocks: scores + exp, then register contributions
                    for i, g in enumerate(needed):
                        full = g < lb  # needed by left half too
                        w = 2 * BLK if full else BLK
                        si_off = 0 if full else BLK
                        for u in range(2):
                            ps = psum.tile([CH, w], F32, tag="ps")
                            for hc in range(HC):
                                nc.tensor.matmul(
                                    ps[:],
                                    kT_all[:, hc, i * BLK + u * CH:i * BLK + (u + 1) * CH],
                                    qT_sb[:, hc, si_off:si_off + w],
                                    start=(hc == 0), stop=(hc == HC - 1))
                            pt = ptpool.tile([CH, 2 * BLK], BF16, tag="pt")
                            nc.scalar.activation(pt[:, 0:w], ps[:],
                                                 mybir.ActivationFunctionType.Exp,
                                                 scale=SCALE)
                            vi = ("gath", 2 * i + u)
                            if full:
                                contribs[0].append((pt, 0, vi))
                                contribs[1].append((pt, CH, vi))
                                contribs[2].append((pt, BLK, vi))
                                contribs[3].append((pt, BLK + CH, vi))
                            else:
                                contribs[2].append((pt, 0, vi))
                                contribs[3].append((pt, CH, vi))

                    # V matmuls: accumulate all contributions per si chunk
                    for sc in range(4):
                        lst = contribs[sc]
                        for j, (pt, col, vi) in enumerate(lst):
                            vt = v_sb[:, vi[1], :] if vi[0] == "loc" else v_all[:, vi[1], :]
                            nc.tensor.matmul(psum_o[sc][:], pt[:, col:col + CH], vt,
                                             start=(j == 0), stop=(j == len(lst) - 1))
                        recip = small.tile([CH, 1], F32, tag=f"recip{sc}")
                        nc.vector.reciprocal(recip[:], psum_o[sc][:, H:H + 1])
                        osb = osbpool.tile([CH, H], F32, tag=f"osb{sc}")
                        nc.vector.tensor_scalar_mul(osb[:], psum_o[sc][:, 0:H], recip[:])
                        nc.sync.dma_start(out[sc * CH:(sc + 1) * CH, :], osb[:])
    nc.compile()
    return nc


def _shard_inputs(marketStateBatch, Wq, bq, Wk, bk, Wv, bv):
    import ml_dtypes
    bf16 = ml_dtypes.bfloat16
    S = marketStateBatch.shape[0]
    BLK = S // N_BLOCKS
    wqT = np.ascontiguousarray(Wq.T.astype(bf16))
    wkT = np.ascontiguousarray(Wk.T.astype(bf16))
    wvT = np.ascontiguousarray(Wv.T.astype(bf16))
    in_maps = []
    for c in range(N_CORES):
        rows = np.r_[c * BLK:(c + 1) * BLK,
                     (N_BLOCKS - 1 - c) * BLK:(N_BLOCKS - c) * BLK]
        xT_c = np.ascontiguousarray(marketStateBatch[rows].T.astype(bf16))
        in_maps.append({
            "xT": xT_c, "wqT": wqT, "wkT": wkT, "wvT": wvT,
            "bq": np.ascontiguousarray(bq[None, :], dtype=np.float32),
            "bk": np.ascontiguousarray(bk[None, :], dtype=np.float32),
            "bv": np.ascontiguousarray(bv[None, :]).astype(bf16),
        })
    return in_maps


def _unshard(results, S):
    BLK = S // N_BLOCKS
    out = np.empty((S, H), dtype=np.float32)
    for c in range(N_CORES):
        r = results[c]["out"]
        out[c * BLK:(c + 1) * BLK] = r[0:BLK]
        out[(N_BLOCKS - 1 - c) * BLK:(N_BLOCKS - c) * BLK] = r[BLK:2 * BLK]
    return out


_NC_CACHE = {}


def kernel(marketStateBatch, Wq, bq, Wk, bk, Wv, bv):
    marketStateBatch = np.asarray(marketStateBatch, dtype=np.float32)
    S, D = marketStateBatch.shape
    key = (S, D)
    if key not in _NC_CACHE:
        _NC_CACHE[key] = build_nc(S, D)
    nc = _NC_CACHE[key]
    in_maps = _shard_inputs(marketStateBatch, np.asarray(Wq), np.asarray(bq),
                            np.asarray(Wk), np.asarray(bk),
                            np.asarray(Wv), np.asarray(bv))
    res = run_bass_kernel_spmd(nc, in_maps, core_ids=list(range(N_CORES)))
    return _unshard(res.results, S)



# revision 4
# speedup vs baseline: 28.8763x; 28.8763x over previous
"""Causal attention head for Trainium2 — single-core, single-packed-input.

Reference computation (single head):
  q = x @ Wq.T + bq ; k = x @ Wk.T + bk ; v = x @ Wv.T + bv
  scores = (q @ k.T) / sqrt(D)  with causal mask, softmax, out = attn @ v

Why one core: per-iteration dispatch through the PJRT relay costs
~0.27 ms per argument buffer PER CORE (serialized), while the whole
problem is ~350 us of silicon on one core. 8-way sharding pays ~4.3 ms
of dispatch to save ~300 us of compute. One core + one packed input
+ one output is the fast configuration.

On-chip schedule (S=4096, D=2048, H=256):
  - All host-side layout prep (x/W transpose, bf16 cast, bias packing)
    is done in _shard_inputs; the device reads one [2049, 4864] bf16
    tensor: cols 0:4096 = x^T, then Wq^T | Wk^T | Wv^T, biases in the
    last row.
  - Projections stream x^T in 4 groups of 1024 rows: q^T/k^T stay in
    [head, seq] layout (so score matmuls contract head-dim on
    partitions); v in [row-chunk, 257] chunks with an all-ones column
    appended (softmax denominator accumulates in PSUM with the
    numerator, no separate reduction).
  - Scores are computed transposed ([key, query]) per 512-wide query
    block, exp'd on the scalar engine (|score| <= 0.66 so no max
    subtraction), causal-masked with affine_select on the <=4 diagonal
    chunks, then attn @ v accumulates per 128-row query chunk.
"""
import sys

sys.path.insert(0, "/opt/trn_rl_repo")

import numpy as np
import concourse.bass as bass
import concourse.mybir as mybir
from concourse import bacc, tile
from concourse.bass_utils import run_bass_kernel_spmd

F32 = mybir.dt.float32
BF16 = mybir.dt.bfloat16
N_CORES = 1
H = 256
HC = 2   # head-dim chunks of 128


def build_nc(S=4096, D=2048):
    KC = D // 128          # contraction chunks (16)
    NB = S // 512          # query blocks (8)
    NQ = S // 128          # query/key chunks (32)
    GROUPS = 4             # x streamed in GROUPS row-groups
    GR = S // GROUPS       # rows per group (1024)
    WCOL = S + 3 * H       # packed width: xT | wqT | wkT | wvT
    SCALE = 1.0 / float(np.sqrt(D))

    nc = bacc.Bacc("TRN2", target_bir_lowering=False, debug=False,
                   enable_asserts=True, num_devices=1)
    inp = nc.dram_tensor("inp", [D + 1, WCOL], BF16, kind="ExternalInput")
    out = nc.dram_tensor("out", [S, H], F32, kind="ExternalOutput")

    with tile.TileContext(nc) as tc:
        with (
            tc.tile_pool(name="w", bufs=1) as wpool,
            tc.tile_pool(name="xg", bufs=2) as xgpool,
            tc.tile_pool(name="qkv", bufs=1) as qkvpool,
            tc.tile_pool(name="small", bufs=1) as small,
            tc.tile_pool(name="pt", bufs=NQ) as ptpool,
            tc.tile_pool(name="osb", bufs=4) as osbpool,
            tc.tile_pool(name="psp", bufs=3, space="PSUM") as psp,
            tc.tile_pool(name="pss", bufs=3, space="PSUM") as pss,
            tc.tile_pool(name="pso", bufs=2, space="PSUM") as pso,
        ):
            # ---- weights + biases -> SBUF ----
            wq_sb = wpool.tile([128, KC, H], BF16, tag="wq")
            wk_sb = wpool.tile([128, KC, H], BF16, tag="wk")
            wv_sb = wpool.tile([128, KC, H], BF16, tag="wv")
            wview = inp[0:D, :].rearrange("(kc p) c -> p kc c", p=128)
            nc.sync.dma_start(wq_sb[:], wview[:, :, S:S + H])
            nc.sync.dma_start(wk_sb[:], wview[:, :, S + H:S + 2 * H])
            nc.sync.dma_start(wv_sb[:], wview[:, :, S + 2 * H:S + 3 * H])
            bqh = small.tile([128, HC], BF16, tag="bqh")
            bkh = small.tile([128, HC], BF16, tag="bkh")
            nc.sync.dma_start(bqh[:], inp[D, 0:H].rearrange("(hc p) -> p hc", p=128))
            nc.sync.dma_start(bkh[:], inp[D, H:2 * H].rearrange("(hc p) -> p hc", p=128))
            bq_sb = small.tile([128, HC], F32, tag="bq")
            bk_sb = small.tile([128, HC], F32, tag="bk")
            nc.vector.tensor_copy(bq_sb[:], bqh[:])
            nc.vector.tensor_copy(bk_sb[:], bkh[:])
            bv1_sb = small.tile([1, H + 1], BF16, tag="bv")
            nc.sync.dma_start(bv1_sb[:, 0:H], inp[D:D + 1, 2 * H:3 * H])
            nc.vector.memset(bv1_sb[:, H:H + 1], 1.0)
            ones_row = small.tile([1, 128], BF16, tag="ones")
            nc.vector.memset(ones_row[:], 1.0)

            # ---- projections, streaming x^T in 4 groups of GR rows ----
            qT_sb = qkvpool.tile([128, HC, S], BF16, tag="qT")
            kT_sb = qkvpool.tile([128, HC, S], BF16, tag="kT")
            v_sb = qkvpool.tile([128, NQ, H + 1], BF16, tag="v")
            xv = inp[0:D, :].rearrange("(kc p) c -> p kc c", p=128)
            for g in range(GROUPS):
                xg = xgpool.tile([128, KC, GR], BF16, tag="xg")
                for k0 in range(0, KC, 4):
                    nc.sync.dma_start(xg[:, k0:k0 + 4, :],
                                      xv[:, k0:k0 + 4, g * GR:(g + 1) * GR])
                # q^T / k^T for this group's columns ([head, seq] layout)
                for hc in range(HC):
                    for half in range(GR // 512):
                        c0 = half * 512
                        ps = psp.tile([128, 512], F32, tag="ps")
                        for kc in range(KC):
                            nc.tensor.matmul(
                                ps[:], wq_sb[:, kc, hc * 128:(hc + 1) * 128],
                                xg[:, kc, c0:c0 + 512],
                                start=(kc == 0), stop=(kc == KC - 1))
                        nc.vector.tensor_scalar_add(
                            qT_sb[:, hc, g * GR + c0:g * GR + c0 + 512],
                            ps[:], bq_sb[:, hc:hc + 1])
                        ps = psp.tile([128, 512], F32, tag="ps")
                        for kc in range(KC):
                            nc.tensor.matmul(
                                ps[:], wk_sb[:, kc, hc * 128:(hc + 1) * 128],
                                xg[:, kc, c0:c0 + 512],
                                start=(kc == 0), stop=(kc == KC - 1))
                        nc.vector.tensor_scalar_add(
                            kT_sb[:, hc, g * GR + c0:g * GR + c0 + 512],
                            ps[:], bk_sb[:, hc:hc + 1])
                # v rows for this group ([row, head] layout, ones col appended)
                for u in range(GR // 128):
                    m = g * (GR // 128) + u
                    ps = psp.tile([128, H + 1], F32, tag="ps")
                    nc.tensor.matmul(ps[:], ones_row[0:1, :], bv1_sb[:],
                                     start=True, stop=False)
                    for kc in range(KC):
                        nc.tensor.matmul(ps[:, 0:H],
                                         xg[:, kc, u * 128:(u + 1) * 128],
                                         wv_sb[:, kc, :],
                                         start=False, stop=(kc == KC - 1))
                    nc.vector.tensor_copy(v_sb[:, m, :], ps[:])

            # ---- causal attention, per 512-wide query block ----
            for b in range(NB):
                nm = 4 * b + 4      # key chunks this block sees
                pts = []
                for m in range(nm):
                    ps = pss.tile([128, 512], F32, tag="ps")
                    for hc in range(HC):
                        nc.tensor.matmul(
                            ps[:], kT_sb[:, hc, m * 128:(m + 1) * 128],
                            qT_sb[:, hc, b * 512:(b + 1) * 512],
                            start=(hc == 0), stop=(hc == HC - 1))
                    pt = ptpool.tile([128, 512], BF16, tag="pt")
                    nc.scalar.activation(pt[:], ps[:],
                                         mybir.ActivationFunctionType.Exp,
                                         scale=SCALE)
                    if m >= 4 * b:
                        # keep iff (col within block) - key_row - (m-4b)*128 >= 0
                        nc.gpsimd.affine_select(
                            out=pt[:], in_=pt[:],
                            compare_op=mybir.AluOpType.is_ge,
                            fill=0.0, base=-(m - 4 * b) * 128,
                            pattern=[[1, 512]], channel_multiplier=-1)
                    pts.append(pt)
                for u in range(4):
                    Q = 4 * b + u   # global 128-row query chunk
                    po = pso.tile([128, H + 1], F32, tag="po")
                    for m in range(Q + 1):
                        nc.tensor.matmul(po[:],
                                         pts[m][:, u * 128:(u + 1) * 128],
                                         v_sb[:, m, :],
                                         start=(m == 0), stop=(m == Q))
                    recip = small.tile([128, 1], F32, tag=f"recip{u}")
                    nc.vector.reciprocal(recip[:], po[:, H:H + 1])
                    osb = osbpool.tile([128, H], F32, tag=f"osb{u}")
                    nc.vector.tensor_scalar_mul(osb[:], po[:, 0:H], recip[:])
                    nc.sync.dma_start(out[Q * 128:(Q + 1) * 128, :], osb[:])
    nc.compile()
    return nc


def _shard_inputs(marketStateBatch, Wq, bq, Wk, bk, Wv, bv):
    import ml_dtypes
    bf16 = ml_dtypes.bfloat16
    S, D = marketStateBatch.shape
    packed = np.zeros((D + 1, S + 3 * H), dtype=bf16)
    packed[0:D, 0:S] = marketStateBatch.T.astype(bf16)
    packed[0:D, S:S + H] = Wq.T.astype(bf16)
    packed[0:D, S + H:S + 2 * H] = Wk.T.astype(bf16)
    packed[0:D, S + 2 * H:S + 3 * H] = Wv.T.astype(bf16)
    packed[D, 0:H] = bq.astype(bf16)
    packed[D, H:2 * H] = bk.astype(bf16)
    packed[D, 2 * H:3 * H] = bv.astype(bf16)
    return [{"inp": packed}]


def _unshard(results, S):
    return results[0]["out"]


_NC_CACHE = {}


def kernel(marketStateBatch, Wq, bq, Wk, bk, Wv, bv):
    marketStateBatch = np.asarray(marketStateBatch, dtype=np.float32)
    S, D = marketStateBatch.shape
    key = (S, D)
    if key not in _NC_CACHE:
        _NC_CACHE[key] = build_nc(S, D)
    nc = _NC_CACHE[key]
    in_maps = _shard_inputs(marketStateBatch, np.asarray(Wq), np.asarray(bq),
                            np.asarray(Wk), np.asarray(bk),
                            np.asarray(Wv), np.asarray(bv))
    res = run_bass_kernel_spmd(nc, in_maps, core_ids=list(range(N_CORES)))
    return _unshard(res.results, S)
